# revision 6
# baseline (speedup 1.0000x reference)
"""Chamfer distance (variant cd-t) Trainium2 kernel.

Problem: x, y: [B=8, dim=3, N=4096] fp32.
  P[b,n,m] = ||x[b,:,n] - y[b,:,m]||^2  (squared euclidean)
  dist_b = mean_n min_m P + mean_m min_n P ; output = mean_b dist_b (scalar fp32)

Sharding: data-parallel over B across 8 NeuronCores (1 batch element/core).

Per-core algorithm:
  The 4096x4096 distance matrix is produced directly in PSUM by a K=5
  augmented matmul:  [x0;x1;x2;1;xx]^T @ [-2y0;-2y1;-2y2;yy;1] = xx + yy - 2 x.y
  Two passes (P and P^T, swapping x/y roles) make BOTH chamfer directions a
  free-axis min-reduce on the Vector engine (no partition-axis min needed).
  Reductions read PSUM in [128, 2048] 4-bank groups to amortize the PSUM
  read overhead.  Tail: pairwise min of the two m-groups, free-axis sum,
  then a 128-partition sum via a ones-vector matmul.  Each core returns its
  two unnormalized sums; the host normalizes and averages over cores.
"""

import numpy as np
from contextlib import ExitStack

import concourse.bass as bass
import concourse.bacc as bacc
import concourse.tile as tile
from concourse import mybir
from concourse.bass_utils import run_bass_kernel_spmd

B, D, N = 8, 3, 4096
K = 5            # augmented contraction dim: [x0,x1,x2,ones,xx]
NT = 128         # n-tile size (PSUM partition dim)
MT = 512         # matmul moving free dim (one fp32 PSUM bank)
GRP = 2048       # vector-reduce group: 4 PSUM banks
F32 = mybir.dt.float32

_cached = {}


def _emit(tc, inp, out):
    nc = tc.nc
    with ExitStack() as ctx:
        const_pool = ctx.enter_context(tc.tile_pool(name="const", bufs=1))
        in_pool = ctx.enter_context(tc.tile_pool(name="inputs", bufs=1))
        psum_pool = ctx.enter_context(tc.tile_pool(name="psum", bufs=2, space="PSUM"))
        red_pool = ctx.enter_context(tc.tile_pool(name="red", bufs=1))

        sb = in_pool.tile([K, 4 * N], F32)
        nc.sync.dma_start(sb[:], inp[:])

        ones = const_pool.tile([128, 1], F32)
        nc.vector.memset(ones[:], 1.0)

        res = red_pool.tile([1, 2], F32, tag="res")

        for p in range(2):           # pass 0: rows of P; pass 1: rows of P^T
            lhs = sb[:, (2 * p) * N:(2 * p) * N + N]          # [K, N] stationary side
            rhs = sb[:, (2 * p + 1) * N:(2 * p + 1) * N + N]  # [K, N] moving side
            minbuf = red_pool.tile([128, 64], F32, tag="minbuf")
            for i in range(N // NT):          # 32 row-tiles
                lhs_i = lhs[:, i * NT:(i + 1) * NT]
                for g in range(N // GRP):     # 2 column groups of 2048
                    pt = psum_pool.tile([128, GRP], F32, tag="pt")
                    for k in range(GRP // MT):    # 4 matmuls, one bank each
                        m0 = g * GRP + k * MT
                        nc.tensor.matmul(
                            pt[:, k * MT:(k + 1) * MT],
                            lhs_i,
                            rhs[:, m0:m0 + MT],
                            start=True, stop=True,
                        )
                    nc.vector.tensor_reduce(
                        minbuf[:, g * 32 + i:g * 32 + i + 1], pt[:],
                        axis=mybir.AxisListType.X, op=mybir.AluOpType.min,
                    )
            # row-min per (partition, row-tile): min of the two column groups
            pairmin = red_pool.tile([128, 32], F32, tag="pairmin")
            nc.vector.tensor_tensor(
                pairmin[:], minbuf[:, 0:32], minbuf[:, 32:64],
                op=mybir.AluOpType.min,
            )
            # sum over the 32 row-tiles (free axis)
            rsum = red_pool.tile([128, 1], F32, tag="rsum")
            nc.vector.tensor_reduce(
                rsum[:], pairmin[:], axis=mybir.AxisListType.X,
                op=mybir.AluOpType.add,
            )
            # sum over 128 partitions: ones^T-style matmul -> PSUM [1,1]
            tp = psum_pool.tile([1, 1], F32, tag="pt")
            nc.tensor.matmul(tp[:], rsum[:], ones[:], start=True, stop=True)
            nc.scalar.copy(res[:, p:p + 1], tp[:])

        nc.sync.dma_start(out[:], res[:])


def _build_program(reps: int = 1):
    if reps in _cached:
        return _cached[reps]
    nc = bacc.Bacc("TRN2", target_bir_lowering=False, debug=False)
    inp = nc.dram_tensor("inp", [K, 4 * N], F32, kind="ExternalInput").ap()
    out = nc.dram_tensor("out", [1, 2], F32, kind="ExternalOutput").ap()
    with tile.TileContext(nc) as tc:
        for _ in range(reps):
            _emit(tc, inp, out)
    nc.compile()
    _cached[reps] = nc
    return nc


def _host_prep(x, y):
    """Build the per-core [K, 4N] augmented input (fp32)."""
    x = np.asarray(x, dtype=np.float32)
    y = np.asarray(y, dtype=np.float32)
    xx = (x * x).sum(axis=0, dtype=np.float32)   # [N]
    yy = (y * y).sum(axis=0, dtype=np.float32)   # [N]
    one = np.ones_like(xx)
    a_l = np.concatenate([x, one[None], xx[None]], axis=0)            # [5, N]
    a_r = np.concatenate([-2.0 * y, yy[None], one[None]], axis=0)     # [5, N]
    b_l = np.concatenate([y, one[None], yy[None]], axis=0)            # [5, N]
    b_r = np.concatenate([-2.0 * x, xx[None], one[None]], axis=0)     # [5, N]
    return np.concatenate([a_l, a_r, b_l, b_r], axis=1)               # [5, 4N]


def kernel(x: np.ndarray, y: np.ndarray) -> np.ndarray:
    nc = _build_program()
    in_maps = [{"inp": _host_prep(x[b], y[b])} for b in range(B)]
    r = run_bass_kernel_spmd(nc, in_maps, core_ids=list(range(B)))
    dists = [(res["out"][0, 0] + res["out"][0, 1]) / np.float32(N)
             for res in r.results]
    return np.float32(sum(dists) / np.float32(B))


# revision 10
# speedup vs baseline: 1.2824x; 1.2824x over previous
"""Chamfer distance (variant cd-t) Trainium2 kernel.

Problem: x, y: [B=8, dim=3, N=4096] fp32.
  P[b,n,m] = ||x[b,:,n] - y[b,:,m]||^2  (squared euclidean)
  dist_b = mean_n min_m P + mean_m min_n P ; output = mean_b dist_b (scalar fp32)

Sharding: data-parallel over B across 8 NeuronCores (1 batch element/core).

Per-core algorithm:
  The 4096x4096 distance matrix is produced directly in PSUM by a K=5
  augmented matmul:  [x0;x1;x2;1;xx]^T @ [-2y0;-2y1;-2y2;yy;1] = xx + yy - 2 x.y
  Two passes (P and P^T, swapping x/y roles) make BOTH chamfer directions a
  free-axis min-reduce on the Vector engine (no partition-axis min needed).
  Reductions read PSUM in [128, 2048] 4-bank groups to amortize the PSUM
  read overhead.  Tail: pairwise min of the two m-groups, free-axis sum,
  then a 128-partition sum via a ones-vector matmul.  Each core returns its
  two unnormalized sums; the host normalizes and averages over cores.
"""

import numpy as np
import ml_dtypes
from contextlib import ExitStack

import concourse.bass as bass
import concourse.bacc as bacc
import concourse.tile as tile
from concourse import mybir
from concourse.bass_utils import run_bass_kernel_spmd

B, D, N = 8, 3, 4096
# Split-bf16 contraction: x,y are split into bf16 (hi, lo) pairs so the whole
# fp32-accurate product fits one bf16 matmul with K=16 rows:
#   [xh(3), xh(3), xl(3), xl(3), 1, 1, xxh, xxl] . [th(3), tl(3), th(3), tl(3), yyh, yyl, 1, 1]
# where t = split(-2y).  bf16 streams 4x faster than fp32 on the PE.
K = 16
NT = 128         # n-tile size (PSUM partition dim)
MT = 512         # matmul moving free dim (one fp32 PSUM bank)
GRP = 2048       # vector-reduce group: 4 PSUM banks
F32 = mybir.dt.float32
BF16 = mybir.dt.bfloat16
NP_BF16 = ml_dtypes.bfloat16

_cached = {}


def _emit(tc, inp, out):
    nc = tc.nc
    with ExitStack() as ctx:
        const_pool = ctx.enter_context(tc.tile_pool(name="const", bufs=1))
        in_pool = ctx.enter_context(tc.tile_pool(name="inputs", bufs=1))
        psum_pool = ctx.enter_context(tc.tile_pool(name="psum", bufs=2, space="PSUM"))
        red_pool = ctx.enter_context(tc.tile_pool(name="red", bufs=1))

        sb = in_pool.tile([K, 4 * N], BF16)
        nc.sync.dma_start(sb[:], inp[:])

        ones = const_pool.tile([128, 1], F32)
        nc.vector.memset(ones[:], 1.0)

        res = red_pool.tile([1, 2], F32, tag="res")

        for p in range(2):           # pass 0: rows of P; pass 1: rows of P^T
            lhs = sb[:, (2 * p) * N:(2 * p) * N + N]          # [K, N] stationary side
            rhs = sb[:, (2 * p + 1) * N:(2 * p + 1) * N + N]  # [K, N] moving side
            minbuf = red_pool.tile([128, 64], F32, tag="minbuf")
            for i in range(N // NT):          # 32 row-tiles
                lhs_i = lhs[:, i * NT:(i + 1) * NT]
                for g in range(N // GRP):     # 2 column groups of 2048
                    pt = psum_pool.tile([128, GRP], F32, tag="pt")
                    for k in range(GRP // MT):    # 4 matmuls, one bank each
                        m0 = g * GRP + k * MT
                        nc.tensor.matmul(
                            pt[:, k * MT:(k + 1) * MT],
                            lhs_i,
                            rhs[:, m0:m0 + MT],
                            start=True, stop=True,
                        )
                    nc.vector.tensor_reduce(
                        minbuf[:, g * 32 + i:g * 32 + i + 1], pt[:],
                        axis=mybir.AxisListType.X, op=mybir.AluOpType.min,
                    )
            # row-min per (partition, row-tile): min of the two column groups
            pairmin = red_pool.tile([128, 32], F32, tag="pairmin")
            nc.vector.tensor_tensor(
                pairmin[:], minbuf[:, 0:32], minbuf[:, 32:64],
                op=mybir.AluOpType.min,
            )
            # sum over the 32 row-tiles (free axis)
            rsum = red_pool.tile([128, 1], F32, tag="rsum")
            nc.vector.tensor_reduce(
                rsum[:], pairmin[:], axis=mybir.AxisListType.X,
                op=mybir.AluOpType.add,
            )
            # sum over 128 partitions: ones^T-style matmul -> PSUM [1,1]
            tp = psum_pool.tile([1, 1], F32, tag="pt")
            nc.tensor.matmul(tp[:], rsum[:], ones[:], start=True, stop=True)
            nc.scalar.copy(res[:, p:p + 1], tp[:])

        nc.sync.dma_start(out[:], res[:])


def _build_program(reps: int = 1):
    if reps in _cached:
        return _cached[reps]
    nc = bacc.Bacc("TRN2", target_bir_lowering=False, debug=False)
    inp = nc.dram_tensor("inp", [K, 4 * N], BF16, kind="ExternalInput").ap()
    out = nc.dram_tensor("out", [1, 2], F32, kind="ExternalOutput").ap()
    with tile.TileContext(nc) as tc:
        for _ in range(reps):
            _emit(tc, inp, out)
    nc.compile()
    _cached[reps] = nc
    return nc


def _split_bf16(a):
    """fp32 [.., N] -> (hi, lo) bf16 pair with hi+lo ~ a to ~2^-17 rel."""
    hi = a.astype(NP_BF16)
    lo = (a - hi.astype(np.float32)).astype(NP_BF16)
    return hi, lo


def _host_prep(x, y):
    """Build the per-core [K=16, 4N] augmented bf16 input."""
    x = np.asarray(x, dtype=np.float32)
    y = np.asarray(y, dtype=np.float32)
    xx = (x * x).sum(axis=0, dtype=np.float32)   # [N]
    yy = (y * y).sum(axis=0, dtype=np.float32)   # [N]
    one = np.ones((1, N), dtype=NP_BF16)

    def stat_rows(ph, pl, sh_, sl_):       # stationary side [16, N]
        return np.concatenate(
            [ph, ph, pl, pl, one, one, sh_[None], sl_[None]], axis=0)

    def mov_rows(th, tl, sh_, sl_):        # moving side [16, N]
        return np.concatenate(
            [th, tl, th, tl, sh_[None], sl_[None], one, one], axis=0)

    xh, xl = _split_bf16(x)                # [3, N] each
    yh, yl = _split_bf16(y)
    t_h, t_l = _split_bf16(-2.0 * y)       # moving side for pass A
    s_h, s_l = _split_bf16(-2.0 * x)       # moving side for pass B
    xxh, xxl = _split_bf16(xx)             # [N]
    yyh, yyl = _split_bf16(yy)

    a_l = stat_rows(xh, xl, xxh, xxl)      # P rows:  stationary = x side
    a_r = mov_rows(t_h, t_l, yyh, yyl)     # moving  = y side
    b_l = stat_rows(yh, yl, yyh, yyl)      # P^T rows
    b_r = mov_rows(s_h, s_l, xxh, xxl)
    return np.concatenate([a_l, a_r, b_l, b_r], axis=1)  # [16, 4N] bf16


def kernel(x: np.ndarray, y: np.ndarray) -> np.ndarray:
    nc = _build_program()
    in_maps = [{"inp": _host_prep(x[b], y[b])} for b in range(B)]
    r = run_bass_kernel_spmd(nc, in_maps, core_ids=list(range(B)))
    dists = [(res["out"][0, 0] + res["out"][0, 1]) / np.float32(N)
             for res in r.results]
    return np.float32(sum(dists) / np.float32(B))
